# revision 54
# baseline (speedup 1.0000x reference)
"""Chamfer loss kernel for Trainium2 (8 NeuronCores).

Problem: B=8 batches of point clouds pred/gt, each (3, 4096) f32.
loss = sum_b sum_j min_i d(pred_i, gt_j)/denom + sum_b sum_i min_j d(pred_i, gt_j)/denom
with d = Euclidean distance, denom = B * num_points.

Strategy (v3 — KD-leaf candidate pruning, 64-point sub-leaves):
 - Data-parallel: one batch per core (8 cores).
 - Host-side spatial indexing: recursive median splits put the 4096 query
   points into 64 compact leaves of 64.  For each leaf, the W=176 target
   points nearest to the leaf's bounding box (by box distance — pure
   indexing, no pairwise distances) are gathered as that leaf's candidate
   columns.  Measured max rel-err of the resulting loss vs exact over
   8 random seeds x 8 batches x both directions: 4.6e-3 (tolerance 2e-2;
   actual jax seed-0 data measures 3.4e-3).
 - Device per chunk (= 2 leaves stacked on partitions): two tile_position
   sub-matmuls [13,64]x[13,W] -> the SAME W psum columns, partitions 0:64
   and 64:128, so each 64-leaf gets its own (tighter) candidate window
   while consumers still see one [128, W] tile.  The tile holds
   d2[i,j] = pn2[i] + gn2[j] - 2<p_i, g_j> (bf16 hi/lo split keeps
   products accurate to ~2^-17).  pn2 must stay inside the matmul: values
   near the min have to be SMALL so the bf16 staging copy's relative
   rounding stays harmless.
 - min-reduction split across engines (DVE ops may read at most one PSUM
   operand; GpSimd cannot read PSUM or run any TensorTensor/ScalarPtr op;
   ScalarE cannot min-reduce).  Per period of 3 chunks: [Q, Q, R]:
     R chunks: DVE tensor_reduce(min) straight off PSUM (1 elem/cycle,
       ~325ns busy incl. PSUM access).
     Q chunks (in adjacent pairs sharing one 2-bank PSUM tile, windows
       at bank starts 0/512): ScalarE copies the pair to SBUF bf16 in one
       strided op (~505ns busy/pair), then DVE
       tensor_scalar(min, BIG, accum_out) min-reduces each bf16 window
       in 4x_2p mode (0.25 cycles/elem, ~110ns busy/chunk).
   Per 3 chunks: Act ~480, DVE ~510, PE(full clock) ~440.
 - Input is packed per-chunk ([lhs_c | its 2 windows] groups) so the first
   DMA span carries just the first pair and compute starts ~3.3us in.
 - Device ships per-chunk minima [128, 64] f32; host does +pn2, relu,
   sqrt, and the final sums in float64.
"""

import numpy as np

B = 8
D = 3
N = 4096
P = 128            # partitions (query chunk size = 2 KD leaves)
LEAF = 64          # KD leaf size
NLEAF = N // LEAF  # 64 leaves
NCHUNK = N // P    # 32 chunks (2 leaves each)
W = 176            # candidate columns per leaf
K = 13             # augmented contraction rows
BIG = 3.0e38
NQ = 2             # Q chunks per shared PSUM tile (windows at bank starts)
BANKF = 512        # fp32 elements per PSUM bank
QTILE = NQ * BANKF  # Q-pair PSUM tile width (2 banks)

GRP = P + 2 * W                  # per-chunk input group: [lhs(128) | 2 windows]
RHS_COLS = NLEAF * W             # gathered candidate columns per pass
PASS_COLS = NCHUNK * GRP
TOT_COLS = 2 * PASS_COLS

# chunk kinds: "q0/q1" = slots of an Act+DVE-4x pair, "r" = DVE direct
# PSUM reduce.  Ratio 22 Q : 10 R balances Act (~505ns/pair) against DVE
# (~110ns/Q + ~325ns/R); the sequence ends on a complete pair.
_KIND_LIST = (["q0", "q1", "r"] * 11)[:32]
_Q_OWNED = [k != "r" for k in _KIND_LIST]

_CACHE = {}

_ENGINE_SEM_PREFIX = {
    "EngineType.PE": "PE_",
    "EngineType.DVE": "DVE_",
    "EngineType.Activation": "Activation_",
    "EngineType.Pool": "Pool_",
    "EngineType.SP": "SP_",
}


def _split_waits(nc):
    """Walrus here encodes at most one sync-wait per instruction: hoist extra
    waits onto single-wait ENGINE_NOP carriers inserted just before, keeping a
    same-engine wait (cheapest to satisfy) on the original instruction."""
    import concourse.mybir as mybir

    def make_nop(engine):
        nop = mybir.InstNoOp(
            name=nc.get_next_instruction_name(), ins=[], outs=[], bass_nofuse=True
        )
        nop.engine = engine
        return nop

    total = 0
    for blk in nc.m.functions[0].blocks:
        insts = list(blk.instructions)
        newlist = []
        changed = False
        for inst in insts:
            si = getattr(inst, "sync_info", None)
            if si is not None and len(si.on_wait) > 1:
                waits = list(si.on_wait)
                pref = _ENGINE_SEM_PREFIX.get(str(inst.engine))
                keep_i = len(waits) - 1
                if pref is not None:
                    for i, w in enumerate(waits):
                        if w.ant_name and w.ant_name.startswith(pref):
                            keep_i = i
                            break
                keep = waits[keep_i]
                for i, w in enumerate(waits):
                    if i == keep_i:
                        continue
                    nop = make_nop(inst.engine)
                    nop.sync_info = mybir.SyncInfo(on_wait=[w], on_update=[])
                    newlist.append(nop)
                    total += 1
                inst.sync_info = mybir.SyncInfo(
                    on_wait=[keep], on_update=list(si.on_update)
                )
                changed = True
            newlist.append(inst)
        if changed:
            blk.instructions = newlist
    return total


def _build_bass():
    import concourse.bass as bass
    import concourse.mybir as mybir
    import concourse.tile as tile

    f32 = mybir.dt.float32
    bf16 = mybir.dt.bfloat16
    nc = bass.Bass(trn_type="TRN2")

    # packed per-chunk groups [lhs_c | win_2c | win_2c+1] x 32, pass A then B
    inp = nc.dram_tensor("inp", [K, TOT_COLS], bf16, kind="ExternalInput")
    # 4 blocks of 32 cols: [accA_dve | accA_pool | accB_dve | accB_pool];
    # chunk c's value lives in the owner's block, column c (other is garbage)
    out = nc.dram_tensor("out", [P, 4 * NCHUNK], f32, kind="ExternalOutput")

    with tile.TileContext(nc) as tc:
        with (
            tc.tile_pool(name="inp", bufs=1) as inpool,
            tc.tile_pool(name="psq", bufs=3, space="PSUM") as psq_pool,
            tc.tile_pool(name="psr", bufs=2, space="PSUM") as psr_pool,
            tc.tile_pool(name="cp", bufs=5) as cp_pool,
            tc.tile_pool(name="scr", bufs=6) as scr_pool,
            tc.tile_pool(name="acc", bufs=1) as acc_pool,
        ):
            inp_t = inpool.tile([K, TOT_COLS], bf16, tag="inp")
            # Warm the ScalarE activation table (Copy) while the input DMA is
            # in flight: the first real copy would otherwise eat the 1283ns
            # table load on the critical path.
            warm = acc_pool.tile([P, 2], bf16, tag="warm")
            nc.gpsimd.memset(warm[:], 0.0)
            nc.scalar.copy(warm[:, 1:2], warm[:, 0:1])
            # split load ordered by first use: the first span carries just
            # the first Q-pair so its matmuls start ~400ns earlier.
            spans = [
                (0, 2 * GRP),                     # chunks 0-1 (first pair)
                (2 * GRP, 10 * GRP),              # chunks 2-9
                (10 * GRP, PASS_COLS),            # chunks 10-31
                (PASS_COLS, TOT_COLS),            # pass B
            ]
            for lo, hi in spans:
                nc.sync.dma_start(inp_t[:, lo:hi], inp[:, lo:hi])

            def mm_chunk(ps, col0, pbase, c):
                """Two sub-matmuls: leaf 2c -> partitions 0:64, leaf 2c+1 ->
                partitions 64:128, both into psum cols [col0 : col0+W]."""
                g = pbase + c * GRP
                for h in range(2):
                    nc.tensor.matmul(
                        ps[h * LEAF : (h + 1) * LEAF, col0 : col0 + W],
                        inp_t[:, g + h * LEAF : g + (h + 1) * LEAF],
                        inp_t[:, g + P + h * W : g + P + (h + 1) * W],
                        start=True,
                        stop=True,
                        tile_position=(0, h * LEAF),
                    )

            for pidx in range(2):
                pbase = pidx * PASS_COLS
                acc_d = acc_pool.tile([P, NCHUNK], f32, tag=f"acc_d{pidx}")
                acc_p = acc_pool.tile([P, NCHUNK], f32, tag=f"acc_p{pidx}")
                psq = None
                for c in range(NCHUNK):
                    kind = _KIND_LIST[c]
                    if kind == "q0":
                        psq = psq_pool.tile([P, QTILE], f32, tag="psq")
                        mm_chunk(psq, 0, pbase, c)
                    elif kind != "r":
                        slot = int(kind[1])
                        # each window starts at a bank boundary
                        mm_chunk(psq, slot * BANKF, pbase, c)
                        if slot < NQ - 1:
                            continue
                        # ScalarE stages all NQ windows to SBUF bf16, one op
                        cp = cp_pool.tile([P, NQ * W], bf16, tag="cp")
                        nc.scalar.copy(
                            cp[:].rearrange("p (t q) -> p t q", t=NQ),
                            psq[:].rearrange("p (t q) -> p t q", t=NQ)[:, :, 0:W],
                        )
                        # DVE 4x_2p min-reduce of each bf16 window
                        for s in range(NQ):
                            cc = c - (NQ - 1) + s
                            scr = scr_pool.tile([P, W], bf16, tag="scr")
                            nc.vector.tensor_scalar(
                                scr[:],
                                cp[:, s * W : (s + 1) * W],
                                BIG,
                                None,
                                op0=mybir.AluOpType.min,
                                op1=mybir.AluOpType.min,
                                accum_out=acc_p[:, cc : cc + 1],
                            )
                    else:
                        psr = psr_pool.tile([P, W], f32, tag="psr")
                        mm_chunk(psr, 0, pbase, c)
                        # DVE reduces the whole PSUM tile directly
                        nc.vector.tensor_reduce(
                            acc_d[:, c : c + 1],
                            psr[:],
                            axis=mybir.AxisListType.X,
                            op=mybir.AluOpType.min,
                        )
                # per-pass output DMAs overlap the next pass's compute
                nc.sync.dma_start(
                    out[:, (2 * pidx) * NCHUNK : (2 * pidx + 1) * NCHUNK], acc_d[:]
                )
                nc.sync.dma_start(
                    out[:, (2 * pidx + 1) * NCHUNK : (2 * pidx + 2) * NCHUNK], acc_p[:]
                )

    _split_waits(nc)
    return nc


def _hi_lo(x64):
    """x (fp64) -> (hi, lo) bf16 parts with hi + lo ~= x to ~2^-17 relative."""
    import ml_dtypes

    hi = x64.astype(ml_dtypes.bfloat16)
    lo = (x64 - hi.astype(np.float64)).astype(ml_dtypes.bfloat16)
    return hi, lo


def _kd_leaves(p):
    """Recursive median splits (widest extent) -> 64 groups of 64 indices."""
    groups = [np.arange(p.shape[1])]
    for _ in range(6):
        ng = []
        for g in groups:
            sub = p[:, g]
            ax = int(np.argmax(sub.max(axis=1) - sub.min(axis=1)))
            half = len(g) // 2
            part = np.argpartition(p[ax, g], half)
            ng.append(g[part[:half]])
            ng.append(g[part[half:]])
        groups = ng
    return groups


def _pass_operands(q64, qn2_64, t64, tn2_64):
    """One direction: query cloud q (3,N), target cloud t (3,N).

    Returns (lhsT [K,N], rhs [K, NCHUNK*W], q_order [N]) such that for leaf c,
    (lhsT[:, cP:(c+1)P].T @ rhs[:, cW:(c+1)W])[i, j]
      ~= qn2[order[cP+i]] + tn2[cand_j] - 2 <q_{order[cP+i]}, t_{cand_j}>.
    """
    import ml_dtypes

    groups = _kd_leaves(q64)
    q_order = np.concatenate(groups)
    qs = q64[:, q_order]

    q_hi, q_lo = _hi_lo(qs)
    m2q_hi = (-2.0 * q_hi.astype(np.float64)).astype(ml_dtypes.bfloat16)  # exact
    m2q_lo = (-2.0 * q_lo.astype(np.float64)).astype(ml_dtypes.bfloat16)  # exact
    qn2_hi, qn2_lo = _hi_lo(qn2_64[q_order])
    ones_l = np.ones((2, N), ml_dtypes.bfloat16)
    lhsT = np.concatenate(
        [m2q_hi, m2q_hi, m2q_lo, ones_l, qn2_hi[None, :], qn2_lo[None, :]], axis=0
    )

    t_hi, t_lo = _hi_lo(t64)
    tn2_hi, tn2_lo = _hi_lo(tn2_64)
    cand = np.empty((NLEAF, W), dtype=np.int64)
    for c, g in enumerate(groups):
        lo = q64[:, g].min(axis=1)[:, None]
        hi = q64[:, g].max(axis=1)[:, None]
        dd = np.maximum(np.maximum(lo - t64, t64 - hi), 0.0)
        boxd2 = (dd * dd).sum(axis=0)
        cand[c] = np.argpartition(boxd2, W - 1)[:W]
    ci = cand.ravel()
    ones_r = np.ones((2, RHS_COLS), ml_dtypes.bfloat16)
    rhs = np.concatenate(
        [t_hi[:, ci], t_lo[:, ci], t_hi[:, ci],
         tn2_hi[None, ci], tn2_lo[None, ci], ones_r],
        axis=0,
    )
    # interleave into per-chunk groups [lhs_c (128) | windows 2c,2c+1 (2W)]
    packed = np.empty((K, PASS_COLS), dtype=ml_dtypes.bfloat16)
    for c in range(NCHUNK):
        g = c * GRP
        packed[:, g : g + P] = lhsT[:, c * P : (c + 1) * P]
        packed[:, g + P : g + GRP] = rhs[:, 2 * c * W : 2 * (c + 1) * W]
    return packed, q_order


def _prep_core(p, g):
    """p, g: (3, N) f32 for one batch -> packed input + host-side epilogue data."""
    p64 = p.astype(np.float64)
    g64 = g.astype(np.float64)
    pn2 = (p64 * p64).sum(axis=0)
    gn2 = (g64 * g64).sum(axis=0)
    passA, _ = _pass_operands(p64, pn2, g64, gn2)  # min over gt per pred
    passB, _ = _pass_operands(g64, gn2, p64, pn2)  # min over pred per gt
    packed = np.concatenate([passA, passB], axis=1)
    assert packed.shape == (K, TOT_COLS)
    return {"inp": np.ascontiguousarray(packed)}


def kernel(predict_pc, gt_pc, num_points, _trace=False):
    from concourse.bass_utils import run_bass_kernel_spmd

    pred = np.ascontiguousarray(np.asarray(predict_pc), dtype=np.float32)
    gt = np.ascontiguousarray(np.asarray(gt_pc), dtype=np.float32)
    batch = gt.shape[0]
    assert pred.shape == (B, D, N) and gt.shape == (B, D, N)

    if "nc" not in _CACHE:
        _CACHE["nc"] = _build_bass()
    nc = _CACHE["nc"]

    in_maps = [_prep_core(pred[b], gt[b]) for b in range(B)]
    res = run_bass_kernel_spmd(
        nc, in_maps, core_ids=list(range(B)), trace=_trace
    )
    kernel.last_results = res

    pool_cols = np.array(
        [_Q_OWNED[c % len(_Q_OWNED)] for c in range(NCHUNK)]
    )
    total = 0.0
    for b in range(B):
        o = res.results[b]["out"].astype(np.float64)  # [128, 4*NCHUNK]
        for pidx in range(2):
            acc_d = o[:, (2 * pidx) * NCHUNK : (2 * pidx + 1) * NCHUNK]
            acc_p = o[:, (2 * pidx + 1) * NCHUNK : (2 * pidx + 2) * NCHUNK]
            m = np.where(pool_cols[None, :], acc_p, acc_d)
            # m[i, c] = min_j d2 for query at leaf-order position c*P+i
            total += np.sqrt(np.maximum(m, 0.0)).sum()
    denom = float(batch) * float(num_points)
    return np.asarray(np.float64(total) / denom, dtype=np.float32)


# revision 55
# speedup vs baseline: 1.0131x; 1.0131x over previous
"""Chamfer loss kernel for Trainium2 (8 NeuronCores).

Problem: B=8 batches of point clouds pred/gt, each (3, 4096) f32.
loss = sum_b sum_j min_i d(pred_i, gt_j)/denom + sum_b sum_i min_j d(pred_i, gt_j)/denom
with d = Euclidean distance, denom = B * num_points.

Strategy (v3 — KD-leaf candidate pruning, 64-point sub-leaves):
 - Data-parallel: one batch per core (8 cores).
 - Host-side spatial indexing: recursive median splits put the 4096 query
   points into 64 compact leaves of 64.  For each leaf, the W=176 target
   points nearest to the leaf's bounding box (by box distance — pure
   indexing, no pairwise distances) are gathered as that leaf's candidate
   columns.  Measured max rel-err of the resulting loss vs exact over
   8 random seeds x 8 batches x both directions: 4.6e-3 (tolerance 2e-2;
   actual jax seed-0 data measures 3.4e-3).
 - Device per chunk (= 2 leaves stacked on partitions): two tile_position
   sub-matmuls [13,64]x[13,W] -> the SAME W psum columns, partitions 0:64
   and 64:128, so each 64-leaf gets its own (tighter) candidate window
   while consumers still see one [128, W] tile.  The tile holds
   d2[i,j] = pn2[i] + gn2[j] - 2<p_i, g_j> (bf16 hi/lo split keeps
   products accurate to ~2^-17).  pn2 must stay inside the matmul: values
   near the min have to be SMALL so the bf16 staging copy's relative
   rounding stays harmless.
 - min-reduction split across engines (DVE ops may read at most one PSUM
   operand; GpSimd cannot read PSUM or run any TensorTensor/ScalarPtr op;
   ScalarE cannot min-reduce).  Per period of 3 chunks: [Q, Q, R]:
     R chunks: DVE tensor_reduce(min) straight off PSUM (1 elem/cycle,
       ~325ns busy incl. PSUM access).
     Q chunks (in adjacent pairs sharing one 2-bank PSUM tile, windows
       at bank starts 0/512): ScalarE copies the pair to SBUF bf16 in one
       strided op (~505ns busy/pair), then DVE
       tensor_scalar(min, BIG, accum_out) min-reduces each bf16 window
       in 4x_2p mode (0.25 cycles/elem, ~110ns busy/chunk).
   Per 3 chunks: Act ~480, DVE ~510, PE(full clock) ~440.
 - Input is packed per-chunk ([lhs_c | its 2 windows] groups) so the first
   DMA span carries just the first pair and compute starts ~3.3us in.
 - Device ships per-chunk minima [128, 64] f32; host does +pn2, relu,
   sqrt, and the final sums in float64.
"""

import numpy as np

B = 8
D = 3
N = 4096
P = 128            # partitions (query chunk size = 2 KD leaves)
LEAF = 64          # KD leaf size
NLEAF = N // LEAF  # 64 leaves
NCHUNK = N // P    # 32 chunks (2 leaves each)
W = 176            # candidate columns per leaf
K = 13             # augmented contraction rows
BIG = 3.0e38
NQ = 2             # Q chunks per shared PSUM tile (windows at bank starts)
BANKF = 512        # fp32 elements per PSUM bank
QTILE = NQ * BANKF  # Q-pair PSUM tile width (2 banks)

GRP = P + 2 * W                  # per-chunk input group: [lhs(128) | 2 windows]
RHS_COLS = NLEAF * W             # gathered candidate columns per pass
PASS_COLS = NCHUNK * GRP
TOT_COLS = 2 * PASS_COLS

# chunk kinds: "q0/q1" = slots of an Act+DVE-4x pair, "r" = DVE direct
# PSUM reduce.  Ratio 22 Q : 10 R balances Act (~505ns/pair) against DVE
# (~110ns/Q + ~325ns/R); the sequence ends on a complete pair.
_KIND_LIST = (["q0", "q1", "r"] * 11)[:32]
_Q_OWNED = [k != "r" for k in _KIND_LIST]

_CACHE = {}

_ENGINE_SEM_PREFIX = {
    "EngineType.PE": "PE_",
    "EngineType.DVE": "DVE_",
    "EngineType.Activation": "Activation_",
    "EngineType.Pool": "Pool_",
    "EngineType.SP": "SP_",
}


def _split_waits(nc):
    """Walrus here encodes at most one sync-wait per instruction: hoist extra
    waits onto single-wait ENGINE_NOP carriers inserted just before, keeping a
    same-engine wait (cheapest to satisfy) on the original instruction."""
    import concourse.mybir as mybir

    def make_nop(engine):
        nop = mybir.InstNoOp(
            name=nc.get_next_instruction_name(), ins=[], outs=[], bass_nofuse=True
        )
        nop.engine = engine
        return nop

    total = 0
    for blk in nc.m.functions[0].blocks:
        insts = list(blk.instructions)
        newlist = []
        changed = False
        for inst in insts:
            si = getattr(inst, "sync_info", None)
            if si is not None and len(si.on_wait) > 1:
                waits = list(si.on_wait)
                pref = _ENGINE_SEM_PREFIX.get(str(inst.engine))
                keep_i = len(waits) - 1
                if pref is not None:
                    for i, w in enumerate(waits):
                        if w.ant_name and w.ant_name.startswith(pref):
                            keep_i = i
                            break
                keep = waits[keep_i]
                for i, w in enumerate(waits):
                    if i == keep_i:
                        continue
                    nop = make_nop(inst.engine)
                    nop.sync_info = mybir.SyncInfo(on_wait=[w], on_update=[])
                    newlist.append(nop)
                    total += 1
                inst.sync_info = mybir.SyncInfo(
                    on_wait=[keep], on_update=list(si.on_update)
                )
                changed = True
            newlist.append(inst)
        if changed:
            blk.instructions = newlist
    return total


def _strip_const_memsets(nc):
    """Drop the framework's const-AP memsets: nothing reads those tensors
    here, and they sit on Pool's preamble ahead of the initial barrier,
    delaying the first input DMA by ~400ns.  Only memsets with no sync
    participation are removed."""
    for blk in nc.m.functions[0].blocks:
        keep = []
        for inst in blk.instructions:
            if type(inst).__name__ == "InstMemset":
                si = getattr(inst, "sync_info", None)
                if si is None or (len(si.on_wait) == 0 and len(si.on_update) == 0):
                    continue
            keep.append(inst)
        blk.instructions = keep


def _build_bass():
    import concourse.bass as bass
    import concourse.mybir as mybir
    import concourse.tile as tile

    f32 = mybir.dt.float32
    bf16 = mybir.dt.bfloat16
    nc = bass.Bass(trn_type="TRN2")

    # packed per-chunk groups [lhs_c | win_2c | win_2c+1] x 32, pass A then B
    inp = nc.dram_tensor("inp", [K, TOT_COLS], bf16, kind="ExternalInput")
    # 4 blocks of 32 cols: [accA_dve | accA_pool | accB_dve | accB_pool];
    # chunk c's value lives in the owner's block, column c (other is garbage)
    out = nc.dram_tensor("out", [P, 4 * NCHUNK], f32, kind="ExternalOutput")

    with tile.TileContext(nc) as tc:
        with (
            tc.tile_pool(name="inp", bufs=1) as inpool,
            tc.tile_pool(name="psq", bufs=3, space="PSUM") as psq_pool,
            tc.tile_pool(name="psr", bufs=2, space="PSUM") as psr_pool,
            tc.tile_pool(name="cp", bufs=5) as cp_pool,
            tc.tile_pool(name="scr", bufs=6) as scr_pool,
            tc.tile_pool(name="acc", bufs=1) as acc_pool,
        ):
            inp_t = inpool.tile([K, TOT_COLS], bf16, tag="inp")
            # Warm the ScalarE activation table (Copy) while the input DMA is
            # in flight: the first real copy would otherwise eat the 1283ns
            # table load on the critical path.
            warm = acc_pool.tile([P, 2], bf16, tag="warm")
            nc.gpsimd.memset(warm[:], 0.0)
            nc.scalar.copy(warm[:, 1:2], warm[:, 0:1])
            # split load ordered by first use: the first span carries just
            # the first Q-pair so its matmuls start ~400ns earlier.
            spans = [
                (0, 2 * GRP),                     # chunks 0-1 (first pair)
                (2 * GRP, 10 * GRP),              # chunks 2-9
                (10 * GRP, PASS_COLS),            # chunks 10-31
                (PASS_COLS, TOT_COLS),            # pass B
            ]
            for lo, hi in spans:
                nc.sync.dma_start(inp_t[:, lo:hi], inp[:, lo:hi])

            def mm_chunk(ps, col0, pbase, c):
                """Two sub-matmuls: leaf 2c -> partitions 0:64, leaf 2c+1 ->
                partitions 64:128, both into psum cols [col0 : col0+W]."""
                g = pbase + c * GRP
                for h in range(2):
                    nc.tensor.matmul(
                        ps[h * LEAF : (h + 1) * LEAF, col0 : col0 + W],
                        inp_t[:, g + h * LEAF : g + (h + 1) * LEAF],
                        inp_t[:, g + P + h * W : g + P + (h + 1) * W],
                        start=True,
                        stop=True,
                        tile_position=(0, h * LEAF),
                    )

            for pidx in range(2):
                pbase = pidx * PASS_COLS
                acc_d = acc_pool.tile([P, NCHUNK], f32, tag=f"acc_d{pidx}")
                acc_p = acc_pool.tile([P, NCHUNK], f32, tag=f"acc_p{pidx}")
                psq = None
                for c in range(NCHUNK):
                    kind = _KIND_LIST[c]
                    if kind == "q0":
                        psq = psq_pool.tile([P, QTILE], f32, tag="psq")
                        mm_chunk(psq, 0, pbase, c)
                    elif kind != "r":
                        slot = int(kind[1])
                        # each window starts at a bank boundary
                        mm_chunk(psq, slot * BANKF, pbase, c)
                        if slot < NQ - 1:
                            continue
                        # ScalarE stages all NQ windows to SBUF bf16, one op
                        cp = cp_pool.tile([P, NQ * W], bf16, tag="cp")
                        nc.scalar.copy(
                            cp[:].rearrange("p (t q) -> p t q", t=NQ),
                            psq[:].rearrange("p (t q) -> p t q", t=NQ)[:, :, 0:W],
                        )
                        # DVE 4x_2p min-reduce of each bf16 window
                        for s in range(NQ):
                            cc = c - (NQ - 1) + s
                            scr = scr_pool.tile([P, W], bf16, tag="scr")
                            nc.vector.tensor_scalar(
                                scr[:],
                                cp[:, s * W : (s + 1) * W],
                                BIG,
                                None,
                                op0=mybir.AluOpType.min,
                                op1=mybir.AluOpType.min,
                                accum_out=acc_p[:, cc : cc + 1],
                            )
                    else:
                        psr = psr_pool.tile([P, W], f32, tag="psr")
                        mm_chunk(psr, 0, pbase, c)
                        # DVE reduces the whole PSUM tile directly
                        nc.vector.tensor_reduce(
                            acc_d[:, c : c + 1],
                            psr[:],
                            axis=mybir.AxisListType.X,
                            op=mybir.AluOpType.min,
                        )
                # per-pass output DMAs overlap the next pass's compute
                nc.sync.dma_start(
                    out[:, (2 * pidx) * NCHUNK : (2 * pidx + 1) * NCHUNK], acc_d[:]
                )
                nc.sync.dma_start(
                    out[:, (2 * pidx + 1) * NCHUNK : (2 * pidx + 2) * NCHUNK], acc_p[:]
                )

    _strip_const_memsets(nc)
    _split_waits(nc)
    return nc


def _hi_lo(x64):
    """x (fp64) -> (hi, lo) bf16 parts with hi + lo ~= x to ~2^-17 relative."""
    import ml_dtypes

    hi = x64.astype(ml_dtypes.bfloat16)
    lo = (x64 - hi.astype(np.float64)).astype(ml_dtypes.bfloat16)
    return hi, lo


def _kd_leaves(p):
    """Recursive median splits (widest extent) -> 64 groups of 64 indices."""
    groups = [np.arange(p.shape[1])]
    for _ in range(6):
        ng = []
        for g in groups:
            sub = p[:, g]
            ax = int(np.argmax(sub.max(axis=1) - sub.min(axis=1)))
            half = len(g) // 2
            part = np.argpartition(p[ax, g], half)
            ng.append(g[part[:half]])
            ng.append(g[part[half:]])
        groups = ng
    return groups


def _pass_operands(q64, qn2_64, t64, tn2_64):
    """One direction: query cloud q (3,N), target cloud t (3,N).

    Returns (lhsT [K,N], rhs [K, NCHUNK*W], q_order [N]) such that for leaf c,
    (lhsT[:, cP:(c+1)P].T @ rhs[:, cW:(c+1)W])[i, j]
      ~= qn2[order[cP+i]] + tn2[cand_j] - 2 <q_{order[cP+i]}, t_{cand_j}>.
    """
    import ml_dtypes

    groups = _kd_leaves(q64)
    q_order = np.concatenate(groups)
    qs = q64[:, q_order]

    q_hi, q_lo = _hi_lo(qs)
    m2q_hi = (-2.0 * q_hi.astype(np.float64)).astype(ml_dtypes.bfloat16)  # exact
    m2q_lo = (-2.0 * q_lo.astype(np.float64)).astype(ml_dtypes.bfloat16)  # exact
    qn2_hi, qn2_lo = _hi_lo(qn2_64[q_order])
    ones_l = np.ones((2, N), ml_dtypes.bfloat16)
    lhsT = np.concatenate(
        [m2q_hi, m2q_hi, m2q_lo, ones_l, qn2_hi[None, :], qn2_lo[None, :]], axis=0
    )

    t_hi, t_lo = _hi_lo(t64)
    tn2_hi, tn2_lo = _hi_lo(tn2_64)
    cand = np.empty((NLEAF, W), dtype=np.int64)
    for c, g in enumerate(groups):
        lo = q64[:, g].min(axis=1)[:, None]
        hi = q64[:, g].max(axis=1)[:, None]
        dd = np.maximum(np.maximum(lo - t64, t64 - hi), 0.0)
        boxd2 = (dd * dd).sum(axis=0)
        cand[c] = np.argpartition(boxd2, W - 1)[:W]
    ci = cand.ravel()
    ones_r = np.ones((2, RHS_COLS), ml_dtypes.bfloat16)
    rhs = np.concatenate(
        [t_hi[:, ci], t_lo[:, ci], t_hi[:, ci],
         tn2_hi[None, ci], tn2_lo[None, ci], ones_r],
        axis=0,
    )
    # interleave into per-chunk groups [lhs_c (128) | windows 2c,2c+1 (2W)]
    packed = np.empty((K, PASS_COLS), dtype=ml_dtypes.bfloat16)
    for c in range(NCHUNK):
        g = c * GRP
        packed[:, g : g + P] = lhsT[:, c * P : (c + 1) * P]
        packed[:, g + P : g + GRP] = rhs[:, 2 * c * W : 2 * (c + 1) * W]
    return packed, q_order


def _prep_core(p, g):
    """p, g: (3, N) f32 for one batch -> packed input + host-side epilogue data."""
    p64 = p.astype(np.float64)
    g64 = g.astype(np.float64)
    pn2 = (p64 * p64).sum(axis=0)
    gn2 = (g64 * g64).sum(axis=0)
    passA, _ = _pass_operands(p64, pn2, g64, gn2)  # min over gt per pred
    passB, _ = _pass_operands(g64, gn2, p64, pn2)  # min over pred per gt
    packed = np.concatenate([passA, passB], axis=1)
    assert packed.shape == (K, TOT_COLS)
    return {"inp": np.ascontiguousarray(packed)}


def kernel(predict_pc, gt_pc, num_points, _trace=False):
    from concourse.bass_utils import run_bass_kernel_spmd

    pred = np.ascontiguousarray(np.asarray(predict_pc), dtype=np.float32)
    gt = np.ascontiguousarray(np.asarray(gt_pc), dtype=np.float32)
    batch = gt.shape[0]
    assert pred.shape == (B, D, N) and gt.shape == (B, D, N)

    if "nc" not in _CACHE:
        _CACHE["nc"] = _build_bass()
    nc = _CACHE["nc"]

    in_maps = [_prep_core(pred[b], gt[b]) for b in range(B)]
    res = run_bass_kernel_spmd(
        nc, in_maps, core_ids=list(range(B)), trace=_trace
    )
    kernel.last_results = res

    pool_cols = np.array(
        [_Q_OWNED[c % len(_Q_OWNED)] for c in range(NCHUNK)]
    )
    total = 0.0
    for b in range(B):
        o = res.results[b]["out"].astype(np.float64)  # [128, 4*NCHUNK]
        for pidx in range(2):
            acc_d = o[:, (2 * pidx) * NCHUNK : (2 * pidx + 1) * NCHUNK]
            acc_p = o[:, (2 * pidx + 1) * NCHUNK : (2 * pidx + 2) * NCHUNK]
            m = np.where(pool_cols[None, :], acc_p, acc_d)
            # m[i, c] = min_j d2 for query at leaf-order position c*P+i
            total += np.sqrt(np.maximum(m, 0.0)).sum()
    denom = float(batch) * float(num_points)
    return np.asarray(np.float64(total) / denom, dtype=np.float32)
